# revision 1
# baseline (speedup 1.0000x reference)
"""Multi-head self-attention (B=8, T=2048, C=192, H=6, HS=32) on 8 TRN2 cores.

Sharding: data-parallel over batch — core i computes batch element i fully
on-chip (no collectives). Host pre-transposes x and packs weights so the
device does zero transposes:

  qT/kT [d, t] = Wq_packed.T @ x.T          (d = h*HS + dd)
  v     [s, d] = x @ Wv_packed, stored per-head as [v_h | ones] (33 cols)
  S^T   [s, t] = kT_h.T @ qT_h              (K=32 matmuls, row-group packed)
  P^T          = exp(S^T / sqrt(HS))        (ScalarE, PSUM->SBUF, bf16)
  [O^T_h; rowsum_h x32] = [v_h|1x32].T @ P^T  (rowsum replicated to rows 32-63)
  OTn_h [d, t] = O^T_h * (1/rowsum_h)       (DVE reciprocal + mul, no bcast)
  out   [t, c] = sum_h OTn_h.T @ Wproj_h + bias  (K=32 accum + rank-1 bias)
"""

import numpy as np
import ml_dtypes
from contextlib import ExitStack

import concourse.bass as bass
import concourse.tile as tile
from concourse import bacc, mybir
from concourse.bass_utils import run_bass_kernel_spmd

B, T, C = 8, 2048, 192
H, HS = 6, 32
P = 128
TCH = 512            # t-chunk width (one PSUM bank of fp32)
NT = T // TCH        # 4
NS = T // P          # 16 s-tiles
SCALE = 1.0 / float(np.sqrt(HS))
BF16 = mybir.dt.bfloat16
F32 = mybir.dt.float32
Exp = mybir.ActivationFunctionType.Exp

_CACHE = {}


def build_nc():
    nc = bacc.Bacc()
    xT = nc.declare_dram_parameter("xT", [C, T], BF16, isOutput=False)
    wq = nc.declare_dram_parameter("wq", [C, H * HS], BF16, isOutput=False)
    wk = nc.declare_dram_parameter("wk", [C, H * HS], BF16, isOutput=False)
    wv = nc.declare_dram_parameter("wv", [C, H * HS], BF16, isOutput=False)
    wp = nc.declare_dram_parameter("wp", [H, HS, C], BF16, isOutput=False)
    bp = nc.declare_dram_parameter("bp", [1, C], BF16, isOutput=False)
    out = nc.declare_dram_parameter("out", [T, C], F32, isOutput=True)

    with tile.TileContext(nc) as tc, ExitStack() as ctx:
        singles = ctx.enter_context(tc.tile_pool(name="singles", bufs=1))
        qk_pool = ctx.enter_context(tc.tile_pool(name="qk", bufs=1))
        vaug_pool = ctx.enter_context(tc.tile_pool(name="vaug", bufs=1))
        pt_pool = ctx.enter_context(tc.tile_pool(name="ptp", bufs=4))
        otn_pool = ctx.enter_context(tc.tile_pool(name="otn", bufs=1))
        small = ctx.enter_context(tc.tile_pool(name="small", bufs=4))
        ysb_pool = ctx.enter_context(tc.tile_pool(name="ysb", bufs=3))

        # ---------------- load inputs ----------------
        xT_a = singles.tile([P, T], BF16)
        nc.sync.dma_start(xT_a, xT[0:P, :])
        xT_b = singles.tile([C - P, T], BF16)
        nc.sync.dma_start(xT_b, xT[P:C, :])

        w_sb = {}
        for name, dram in (("q", wq), ("k", wk), ("v", wv)):
            a = singles.tile([P, H * HS], BF16, name=f"w{name}a")
            nc.sync.dma_start(a, dram[0:P, :])
            b = singles.tile([C - P, H * HS], BF16, name=f"w{name}b")
            nc.sync.dma_start(b, dram[P:C, :])
            w_sb[name] = (a, b)

        wp_sb = []
        for h in range(H):
            wph = singles.tile([HS, C], BF16, name=f"wp{h}")
            nc.sync.dma_start(wph, wp[h, :, :])
            wp_sb.append(wph)
        bp_sb = singles.tile([1, C], BF16)
        nc.sync.dma_start(bp_sb, bp[:, :])
        ones1 = singles.tile([1, P], BF16)
        nc.vector.memset(ones1, 1.0)

        # ---------------- phase 1: qT, kT, v_aug ----------------
        qT_a = qk_pool.tile([P, T], BF16)       # heads 0..3, d-major
        qT_b = qk_pool.tile([C - P, T], BF16)   # heads 4,5
        kT_a = qk_pool.tile([P, T], BF16)
        kT_b = qk_pool.tile([C - P, T], BF16)
        v_aug = []
        with tc.tile_pool(name="pqkv", bufs=2, space="PSUM") as pqkv:
            for proj, dst_a, dst_b in (("q", qT_a, qT_b), ("k", kT_a, kT_b)):
                wa, wb = w_sb[proj]
                for dlo, dsz, dst in ((0, P, dst_a), (P, C - P, dst_b)):
                    for t0 in range(0, T, TCH):
                        ps = pqkv.tile([P, TCH], F32, name="psq", tag="psq")
                        nc.tensor.matmul(
                            ps[0:dsz, :], wa[:, dlo:dlo + dsz],
                            xT_a[:, t0:t0 + TCH], start=True, stop=False)
                        nc.tensor.matmul(
                            ps[0:dsz, :], wb[:, dlo:dlo + dsz],
                            xT_b[:, t0:t0 + TCH], start=False, stop=True)
                        nc.vector.tensor_copy(
                            dst[0:dsz, t0:t0 + TCH], ps[0:dsz, :])
            wva, wvb = w_sb["v"]
            for si in range(NS):
                s0 = si * P
                va = vaug_pool.tile(
                    [P, H * 2 * HS], BF16, name=f"vaug{si}", tag=f"vaug{si}")
                ps = pqkv.tile([P, H * HS], F32, name="psv", tag="psv")
                nc.tensor.matmul(ps, xT_a[:, s0:s0 + P], wva,
                                 start=True, stop=False)
                nc.tensor.matmul(ps, xT_b[:, s0:s0 + P], wvb,
                                 start=False, stop=True)
                va_r = va.rearrange("p (h e) -> p h e", h=H)
                ps_r = ps.rearrange("p (h d) -> p h d", h=H)
                nc.vector.tensor_copy(va_r[:, :, 0:HS], ps_r)
                nc.vector.memset(va_r[:, :, HS:2 * HS], 1.0)
                v_aug.append(va)

        # ---------------- phase 2: attention ----------------
        otn = [otn_pool.tile([HS, T], BF16, name=f"otn{h}", tag=f"otn{h}")
               for h in range(H)]
        # head pairs (A=2p, B=2p+1); within a pair kT/qT rows sit in
        # distinct 32-row groups, so the two QKT matmuls run concurrently
        def hsrc(h):
            if h < 4:
                return kT_a, qT_a, HS * h
            return kT_b, qT_b, HS * (h - 4)
        with (
            tc.tile_pool(name="pst", bufs=2, space="PSUM") as pst_pool,
            tc.tile_pool(name="pav", bufs=1, space="PSUM") as pav_pool,
            tc.tile_pool(name="py", bufs=1, space="PSUM") as py_pool,
        ):
            for tc0 in range(0, T, TCH):
                av = [pav_pool.tile([P, TCH], F32,
                                    name=f"avp{p}", tag=f"avp{p}")
                      for p in range(H // 2)]
                for si in range(NS):
                    s0 = si * P
                    for p in range(H // 2):
                        hA, hB = 2 * p, 2 * p + 1
                        stp = pst_pool.tile([P, 2 * TCH], F32,
                                            name="stp", tag="stp")
                        for half, h in ((0, hA), (1, hB)):
                            kT_t, qT_t, pb = hsrc(h)
                            nc.tensor.matmul(
                                stp[:, half * TCH:(half + 1) * TCH],
                                kT_t[pb:pb + HS, s0:s0 + P],
                                qT_t[pb:pb + HS, tc0:tc0 + TCH],
                                start=True, stop=True, tile_position=(pb, 0))
                        ptp = pt_pool.tile([P, 2 * TCH], BF16,
                                           name="ptp", tag="ptp")
                        nc.scalar.activation(ptp, stp, Exp, scale=SCALE)
                        for half, h in ((0, hA), (1, hB)):
                            nc.tensor.matmul(
                                av[p][64 * half:64 * half + 64, :],
                                v_aug[si][:, 2 * HS * h:2 * HS * (h + 1)],
                                ptp[:, half * TCH:(half + 1) * TCH],
                                start=(si == 0), stop=(si == NS - 1),
                                skip_group_check=True,
                                tile_position=(0, 64 * half))
                for p in range(H // 2):
                    rbp = small.tile([P, TCH], F32, name="rbp", tag="rbp")
                    for half, h in ((0, 2 * p), (1, 2 * p + 1)):
                        b = 64 * half
                        nc.vector.reciprocal(
                            rbp[b:b + HS, :], av[p][b + HS:b + 2 * HS, :])
                        nc.vector.tensor_mul(
                            otn[h][:, tc0:tc0 + TCH],
                            av[p][b:b + HS, :], rbp[b:b + HS, :])
                # ---- projection for this t-chunk (spare PSUM bank) ----
                for tt in range(tc0, tc0 + TCH, P):
                    ps = py_pool.tile([P, C], F32, name="psy", tag="psy")
                    nc.tensor.matmul(ps, ones1, bp_sb, start=True, stop=False)
                    for h in range(H):
                        nc.tensor.matmul(
                            ps, otn[h][:, tt:tt + P], wp_sb[h],
                            start=False, stop=(h == H - 1))
                    ysb = ysb_pool.tile([P, C], F32, name="ysbt", tag="ysbt")
                    nc.vector.tensor_copy(ysb, ps)
                    nc.sync.dma_start(out[tt:tt + P, :], ysb)

    nc.compile()
    return nc


def _get_nc():
    if "nc" not in _CACHE:
        _CACHE["nc"] = build_nc()
    return _CACHE["nc"]


def make_in_maps(x, Wq, Wk, Wv, Wproj, bproj):
    bf = ml_dtypes.bfloat16
    x = np.asarray(x, np.float32)
    pack = lambda w: np.ascontiguousarray(
        np.transpose(np.asarray(w, np.float32), (1, 0, 2)).reshape(C, H * HS)
    ).astype(bf)
    wq, wk, wv = pack(Wq), pack(Wk), pack(Wv)
    wp = np.ascontiguousarray(
        np.asarray(Wproj, np.float32).reshape(H, HS, C)).astype(bf)
    bp = np.asarray(bproj, np.float32).reshape(1, C).astype(bf)
    maps = []
    for i in range(B):
        xti = np.ascontiguousarray(x[i].T).astype(bf)
        maps.append({"xT": xti, "wq": wq, "wk": wk, "wv": wv,
                     "wp": wp, "bp": bp})
    return maps


def run(inputs, trace=False, **kw):
    nc = _get_nc()
    in_maps = make_in_maps(**inputs)
    res = run_bass_kernel_spmd(nc, in_maps, core_ids=list(range(B)),
                               trace=trace, **kw)
    y = np.stack([np.asarray(res.results[i]["out"], np.float32)
                  for i in range(B)], axis=0)
    return y, res


def kernel(**inputs):
    y, _ = run(inputs, trace=False)
    return y



# revision 5
# speedup vs baseline: 1.3262x; 1.3262x over previous
"""Multi-head self-attention (B=8, T=2048, C=192, H=6, HS=32) on 8 TRN2 cores.

Sharding: data-parallel over batch - core i computes batch element i fully
on-chip (no collectives).

v2 design (cost model: matmul = out-free-cols * pe_cycle; ACT/DVE/Pool =
free-cols * engine cycle):
  - qT/kT [d, t] projections as in v1 (PSUM->SBUF copies on ACT).
  - v_aug [s, 6*33] = x @ Wv_aug with a built-in ones column per head
    (host pads xT with a ones row; Wv_aug carries the ones pattern).
  - S^T [s, t] per head: 1 matmul per (head, s-tile, t-chunk), K=32.
  - P^T = exp(S^T/sqrt(HS)) split across three engines:
      ACT:  exact activation-Exp
      DVE:  Schraudolph bf16-bits exp (int16(A*x+B) bitcast to bf16)
      Pool: same trick from an fp16 staging copy made by ACT/DVE
            (GPSIMD cannot read PSUM)
  - AV in [t, d] orientation: out [t-tile 128, 33] per (t-tile, head,
    s-block) accumulated over 16 s-blocks; col 32 = rowsum via the ones col.
  - normalize via DVE broadcast mul, ones col appended -> On [128, 193] f32
  - PE transpose (fp32, via identity) -> O^T in reused AV psum banks,
    ACT/DVE copy to SBUF -> otnT_a [97, 128] (row 96 = softmax ones ->
    bias via Wp_a row 96 = bproj), otnT_b [96, 128]
  - out projection: 2 matmuls (K=97/96) + DVE copy + DMA per t-tile.
"""

import numpy as np
import ml_dtypes
from contextlib import ExitStack

import concourse.bass as bass
import concourse.tile as tile
from concourse import bacc, mybir
from concourse.bass_utils import run_bass_kernel_spmd

B, T, C = 8, 2048, 192
H, HS = 6, 32
P = 128
TCH = 512            # t-chunk width of one S^T tile pair
NT = T // TCH        # 4
NS = T // P          # 16 s-tiles / t-tiles
SCALE = 1.0 / float(np.sqrt(HS))
BF16 = mybir.dt.bfloat16
F16 = mybir.dt.float16
F32 = mybir.dt.float32
I16 = mybir.dt.int16
Exp = mybir.ActivationFunctionType.Exp
Copy = mybir.ActivationFunctionType.Copy
MUL = mybir.AluOpType.mult
ADD = mybir.AluOpType.add

# Schraudolph constants for bf16-bits exp: bf16_bits = int16(A*x + B)
SCH_A = 128.0 / np.log(2.0)
SCH_B = 127.0 * 128.0 - 7.5 + 0.5

# exp-engine split (192 tiles total): ACT direct / DVE direct / Pool via
# fp16 staging; pool staging copies alternate ACT/DVE with given counts.
N_ACT, N_DVE, N_POOL = 67, 54, 71
N_POOLCP_ACT = 40     # of the pool tiles, how many staging copies ACT makes

_CACHE = {}


def _exp_plan():
    """Weighted round-robin: list of (engine, copier) for the 192 exp tiles."""
    plan = []
    acc = {"act": 0.0, "dve": 0.0, "pool": 0.0}
    w = {"act": N_ACT / 192.0, "dve": N_DVE / 192.0, "pool": N_POOL / 192.0}
    n = {"act": 0, "dve": 0, "pool": 0}
    cap = {"act": N_ACT, "dve": N_DVE, "pool": N_POOL}
    cp_acc, cp_n = 0.0, 0
    for _ in range(192):
        for k in acc:
            acc[k] += w[k]
        pick = max((k for k in acc if n[k] < cap[k]), key=lambda k: acc[k])
        acc[pick] -= 1.0
        n[pick] += 1
        copier = None
        if pick == "pool":
            cp_acc += N_POOLCP_ACT / float(N_POOL)
            if cp_acc >= 1.0 and cp_n < N_POOLCP_ACT:
                cp_acc -= 1.0
                cp_n += 1
                copier = "act"
            else:
                copier = "dve"
        plan.append((pick, copier))
    return plan


def build_nc():
    nc = bacc.Bacc()
    xT = nc.declare_dram_parameter("xT", [C + 1, T], BF16, isOutput=False)
    wq = nc.declare_dram_parameter("wq", [C, H * HS], BF16, isOutput=False)
    wk = nc.declare_dram_parameter("wk", [C, H * HS], BF16, isOutput=False)
    wv = nc.declare_dram_parameter("wv", [C + 1, H * 33], BF16, isOutput=False)
    wpa = nc.declare_dram_parameter("wpa", [97, C], BF16, isOutput=False)
    wpb = nc.declare_dram_parameter("wpb", [96, C], BF16, isOutput=False)
    idn = nc.declare_dram_parameter("idn", [P, P], F32, isOutput=False)
    out = nc.declare_dram_parameter("out", [T, C], F32, isOutput=True)

    plan = _exp_plan()

    with tile.TileContext(nc) as tc, ExitStack() as ctx:
        singles = ctx.enter_context(tc.tile_pool(name="singles", bufs=1))
        qk_pool = ctx.enter_context(tc.tile_pool(name="qk", bufs=1))
        vaug_pool = ctx.enter_context(tc.tile_pool(name="vaug", bufs=1))
        pt_pool = ctx.enter_context(tc.tile_pool(name="ptp", bufs=12))
        stg_pool = ctx.enter_context(tc.tile_pool(name="stg", bufs=4))
        on_pool = ctx.enter_context(tc.tile_pool(name="onp", bufs=4))
        ot_pool = ctx.enter_context(tc.tile_pool(name="otp", bufs=2))
        small = ctx.enter_context(tc.tile_pool(name="small", bufs=4))
        ysb_pool = ctx.enter_context(tc.tile_pool(name="ysb", bufs=3))

        # ---------------- load inputs ----------------
        xa = singles.tile([P, T], BF16)
        nc.sync.dma_start(xa, xT[0:P, :])
        xb = singles.tile([C + 1 - P, T], BF16)          # 65 rows (ones last)
        nc.sync.dma_start(xb, xT[P:C + 1, :])

        w_sb = {}
        for name, dram in (("q", wq), ("k", wk)):
            a = singles.tile([P, H * HS], BF16, name=f"w{name}a")
            nc.sync.dma_start(a, dram[0:P, :])
            b = singles.tile([C - P, H * HS], BF16, name=f"w{name}b")
            nc.sync.dma_start(b, dram[P:C, :])
            w_sb[name] = (a, b)
        wva = singles.tile([P, H * 33], BF16)
        nc.sync.dma_start(wva, wv[0:P, :])
        wvb = singles.tile([C + 1 - P, H * 33], BF16)
        nc.sync.dma_start(wvb, wv[P:C + 1, :])
        wpa_sb = singles.tile([97, C], BF16)
        nc.sync.dma_start(wpa_sb, wpa[:, :])
        wpb_sb = singles.tile([96, C], BF16)
        nc.sync.dma_start(wpb_sb, wpb[:, :])
        idn_sb = singles.tile([P, P], F32)
        nc.sync.dma_start(idn_sb, idn[:, :])

        # ---------------- phase 1: qT, kT, v_aug ----------------
        qT_a = qk_pool.tile([P, T], BF16)       # heads 0..3, d-major
        qT_b = qk_pool.tile([C - P, T], BF16)   # heads 4,5
        kT_a = qk_pool.tile([P, T], BF16)
        kT_b = qk_pool.tile([C - P, T], BF16)
        v_aug = []
        with tc.tile_pool(name="pqk", bufs=2, space="PSUM") as pqk, \
             tc.tile_pool(name="pv", bufs=2, space="PSUM") as pv:
            for proj, dst_a, dst_b in (("q", qT_a, qT_b), ("k", kT_a, kT_b)):
                wa, wb = w_sb[proj]
                for dlo, dsz, dst in ((0, P, dst_a), (P, C - P, dst_b)):
                    for t0 in range(0, T, TCH):
                        ps = pqk.tile([P, TCH], F32, name="psq", tag="psq")
                        nc.tensor.matmul(
                            ps[0:dsz, :], wa[:, dlo:dlo + dsz],
                            xa[:, t0:t0 + TCH], start=True, stop=False)
                        nc.tensor.matmul(
                            ps[0:dsz, :], wb[:, dlo:dlo + dsz],
                            xb[0:C - P, t0:t0 + TCH], start=False, stop=True)
                        nc.scalar.activation(
                            dst[0:dsz, t0:t0 + TCH], ps[0:dsz, :], Copy,
                            scale=1.0)
            for si in range(NS):
                s0 = si * P
                ps = pv.tile([P, H * 33], F32, name="psv", tag="psv")
                nc.tensor.matmul(ps, xa[:, s0:s0 + P], wva,
                                 start=True, stop=False)
                nc.tensor.matmul(ps, xb[:, s0:s0 + P], wvb,
                                 start=False, stop=True)
                va = vaug_pool.tile([P, H * 33], BF16,
                                    name=f"vaug{si}", tag=f"vaug{si}")
                nc.vector.tensor_copy(va, ps)
                v_aug.append(va)

        # ---------------- phase 2: attention ----------------
        def hsrc(h):
            if h < 4:
                return kT_a, qT_a, HS * h
            return kT_b, qT_b, HS * (h - 4)

        exp_i = [0]

        with (
            tc.tile_pool(name="pst", bufs=3, space="PSUM") as pst_pool,
            tc.tile_pool(name="pav", bufs=1, space="PSUM") as pav_pool,
        ):
            for tci, tc0 in enumerate(range(0, T, TCH)):
                # two av accumulators, each one PSUM bank: cols =
                # (tt%2)*198 + h*33 + [0..32]; col 32 of each head = rowsum
                av = [pav_pool.tile([P, 396], F32, name=f"av{b}", tag=f"av{b}")
                      for b in range(2)]
                pend = {}

                def issue_av(si):
                    for tt in range(4):
                        b, jj = tt // 2, tt % 2
                        for h in range(H):
                            p, half = h // 2, h % 2
                            ptp = pend[si][p]
                            nc.tensor.matmul(
                                av[b][:, jj * 198 + h * 33:
                                      jj * 198 + (h + 1) * 33],
                                ptp[:, half * TCH + tt * P:
                                    half * TCH + tt * P + P],
                                v_aug[si][:, h * 33:(h + 1) * 33],
                                # start=True marks the WHOLE psum bank
                                # pending-zero, so only the first chain into
                                # each bank may set it; the other chains'
                                # first writes then overwrite pending-zero
                                # bytes (= implicit zero init).
                                start=(si == 0 and jj == 0 and h == 0),
                                stop=(si == NS - 1),
                                skip_group_check=True)

                for si in range(NS):
                    s0 = si * P
                    if si >= 2:
                        issue_av(si - 2)
                    cur = []
                    for p in range(H // 2):
                        hA, hB = 2 * p, 2 * p + 1
                        stp = pst_pool.tile([P, 2 * TCH], F32,
                                            name="stp", tag="stp")
                        for half, h in ((0, hA), (1, hB)):
                            kT_t, qT_t, pb = hsrc(h)
                            nc.tensor.matmul(
                                stp[:, half * TCH:(half + 1) * TCH],
                                kT_t[pb:pb + HS, s0:s0 + P],
                                qT_t[pb:pb + HS, tc0:tc0 + TCH],
                                start=True, stop=True, tile_position=(pb, 0))
                        eng, copier = plan[exp_i[0]]
                        exp_i[0] += 1
                        ptp = pt_pool.tile([P, 2 * TCH], BF16,
                                           name="ptp", tag="ptp")
                        if eng == "act":
                            nc.scalar.activation(ptp, stp, Exp, scale=SCALE)
                        elif eng == "dve":
                            nc.vector.tensor_scalar(
                                ptp.bitcast(I16), stp, SCH_A * SCALE, SCH_B,
                                op0=MUL, op1=ADD)
                        else:
                            stg = stg_pool.tile([P, 2 * TCH], F16,
                                                name="stg", tag="stg")
                            if copier == "act":
                                nc.scalar.activation(stg, stp, Copy,
                                                     scale=SCALE)
                            else:
                                nc.vector.tensor_scalar_mul(stg, stp, SCALE)
                            nc.gpsimd.tensor_scalar(
                                ptp.bitcast(I16), stg, SCH_A, SCH_B,
                                op0=MUL, op1=ADD)
                        cur.append(ptp)
                    pend[si] = cur
                issue_av(NS - 2)
                issue_av(NS - 1)

                # ---- tail: normalize all 4 t-tiles first (frees av banks)
                rrecs = []
                ons = []
                for tt in range(4):
                    b, off = tt // 2, (tt % 2) * 198
                    avr = av[b][:, off:off + 198].rearrange(
                        "p (h e) -> p h e", h=H)
                    rrec = small.tile([P, H], F32, name="rrec", tag="rrec")
                    nc.vector.reciprocal(rrec[:, :, None], avr[:, :, 32:33])
                    on = on_pool.tile([P, 193], F32, name="on", tag="on")
                    for g in range(2):
                        og = 97 * g
                        nc.vector.tensor_tensor(
                            on[:, og:og + 96].rearrange(
                                "p (h e) -> p h e", h=3),
                            avr[:, 3 * g:3 * g + 3, 0:32],
                            rrec[:, 3 * g:3 * g + 3, None].to_broadcast(
                                (P, 3, 32)),
                            op=MUL)
                    nc.gpsimd.memset(on[:, 96:97], 1.0)
                    rrecs.append(rrec)
                    ons.append(on)

                # ---- transpose + project, reusing the freed av banks
                # av[0] holds three 128-col transpose slots (rotating),
                # av[1] holds two 192-col projection-psum slots.
                for tt in range(4):
                    on = ons[tt]
                    ca = 128 * ((2 * tt) % 3)
                    cb = 128 * ((2 * tt + 1) % 3)
                    ga = av[0][:, ca:ca + 128]
                    gb = av[0][:, cb:cb + 128]
                    nc.tensor.transpose(ga[0:97, :], on[:, 0:97], idn_sb)
                    ota = ot_pool.tile([97, P], BF16, name="ota", tag="ota")
                    nc.scalar.activation(ota, ga[0:97, :], Copy, scale=1.0)
                    nc.tensor.transpose(gb[0:96, :], on[:, 97:193], idn_sb)
                    otb = ot_pool.tile([96, P], BF16, name="otb", tag="otb")
                    nc.vector.tensor_copy(otb, gb[0:96, :])
                    py = av[1][:, (tt % 2) * 192:(tt % 2) * 192 + 192]
                    nc.tensor.matmul(py, ota, wpa_sb, start=True, stop=False,
                                     skip_group_check=True)
                    nc.tensor.matmul(py, otb, wpb_sb, start=False, stop=True,
                                     skip_group_check=True)
                    ysb = ysb_pool.tile([P, C], F32, name="ysbt", tag="ysbt")
                    nc.vector.tensor_copy(ysb, py)
                    nc.sync.dma_start(out[tc0 + tt * P:tc0 + (tt + 1) * P, :],
                                      ysb)

    nc.compile()
    return nc


def _get_nc():
    if "nc" not in _CACHE:
        _CACHE["nc"] = build_nc()
    return _CACHE["nc"]


def make_in_maps(x, Wq, Wk, Wv, Wproj, bproj):
    bf = ml_dtypes.bfloat16
    x = np.asarray(x, np.float32)
    pack = lambda w: np.ascontiguousarray(
        np.transpose(np.asarray(w, np.float32), (1, 0, 2)).reshape(C, H * HS)
    ).astype(bf)
    wq, wk = pack(Wq), pack(Wk)

    wv_aug = np.zeros((C + 1, H * 33), np.float32)
    Wv = np.asarray(Wv, np.float32)
    for h in range(H):
        wv_aug[0:C, h * 33:h * 33 + 32] = Wv[h]
        wv_aug[C, h * 33 + 32] = 1.0
    wv_aug = wv_aug.astype(bf)

    Wp = np.asarray(Wproj, np.float32)          # [H*HS, C]
    wpa = np.zeros((97, C), np.float32)
    wpa[0:96] = Wp[0:96]
    wpa[96] = np.asarray(bproj, np.float32)
    wpb = Wp[96:192]
    wpa = wpa.astype(bf)
    wpb = np.ascontiguousarray(wpb).astype(bf)

    idn = np.eye(P, dtype=np.float32)

    maps = []
    for i in range(B):
        xp = np.ones((C + 1, T), np.float32)
        xp[0:C] = x[i].T
        maps.append({"xT": xp.astype(bf), "wq": wq, "wk": wk,
                     "wv": wv_aug, "wpa": wpa, "wpb": wpb, "idn": idn})
    return maps


def run(inputs, trace=False, **kw):
    nc = _get_nc()
    in_maps = make_in_maps(**inputs)
    res = run_bass_kernel_spmd(nc, in_maps, core_ids=list(range(B)),
                               trace=trace, **kw)
    y = np.stack([np.asarray(res.results[i]["out"], np.float32)
                  for i in range(B)], axis=0)
    return y, res


def kernel(**inputs):
    y, _ = run(inputs, trace=False)
    return y


# revision 6
# speedup vs baseline: 1.4642x; 1.1040x over previous
"""Multi-head self-attention (B=8, T=2048, C=192, H=6, HS=32) on 8 TRN2 cores.

Sharding: data-parallel over batch - core i computes batch element i fully
on-chip (no collectives).

v2 design (cost model: matmul = out-free-cols * pe_cycle; ACT/DVE/Pool =
free-cols * engine cycle):
  - qT/kT [d, t] projections as in v1 (PSUM->SBUF copies on ACT).
  - v_aug [s, 6*33] = x @ Wv_aug with a built-in ones column per head
    (host pads xT with a ones row; Wv_aug carries the ones pattern).
  - S^T [s, t] per head: 1 matmul per (head, s-tile, t-chunk), K=32.
  - P^T = exp(S^T/sqrt(HS)) split across three engines:
      ACT:  exact activation-Exp
      DVE:  Schraudolph bf16-bits exp (int16(A*x+B) bitcast to bf16)
      Pool: same trick from an fp16 staging copy made by ACT/DVE
            (GPSIMD cannot read PSUM)
  - AV in [t, d] orientation: out [t-tile 128, 33] per (t-tile, head,
    s-block) accumulated over 16 s-blocks; col 32 = rowsum via the ones col.
  - normalize via DVE broadcast mul, ones col appended -> On [128, 193] f32
  - PE transpose (fp32, via identity) -> O^T in reused AV psum banks,
    ACT/DVE copy to SBUF -> otnT_a [97, 128] (row 96 = softmax ones ->
    bias via Wp_a row 96 = bproj), otnT_b [96, 128]
  - out projection: 2 matmuls (K=97/96) + DVE copy + DMA per t-tile.
"""

import numpy as np
import ml_dtypes
from contextlib import ExitStack

import concourse.bass as bass
import concourse.tile as tile
from concourse import bacc, mybir
from concourse.bass_utils import run_bass_kernel_spmd

B, T, C = 8, 2048, 192
H, HS = 6, 32
P = 128
TCH = 512            # t-chunk width of one S^T tile pair
NT = T // TCH        # 4
NS = T // P          # 16 s-tiles / t-tiles
SCALE = 1.0 / float(np.sqrt(HS))
BF16 = mybir.dt.bfloat16
F16 = mybir.dt.float16
F32 = mybir.dt.float32
I16 = mybir.dt.int16
Exp = mybir.ActivationFunctionType.Exp
Copy = mybir.ActivationFunctionType.Copy
MUL = mybir.AluOpType.mult
ADD = mybir.AluOpType.add

# Schraudolph constants for bf16-bits exp: bf16_bits = int16(A*x + B)
SCH_A = 128.0 / np.log(2.0)
SCH_B = 127.0 * 128.0 - 7.5 + 0.5

# exp-engine split (192 tiles total): ACT direct / DVE direct / Pool via
# fp16 staging. Pool exp is a net loss (the PSUM->SBUF staging copy costs
# the copier engine as much as doing the exp directly), so N_POOL=0.
N_ACT, N_DVE, N_POOL = 107, 85, 0
N_POOLCP_ACT = 0      # of the pool tiles, how many staging copies ACT makes

_CACHE = {}


def _exp_plan():
    """Weighted round-robin: list of (engine, copier) for the 192 exp tiles."""
    plan = []
    acc = {"act": 0.0, "dve": 0.0, "pool": 0.0}
    w = {"act": N_ACT / 192.0, "dve": N_DVE / 192.0, "pool": N_POOL / 192.0}
    n = {"act": 0, "dve": 0, "pool": 0}
    cap = {"act": N_ACT, "dve": N_DVE, "pool": N_POOL}
    cp_acc, cp_n = 0.0, 0
    for _ in range(192):
        for k in acc:
            acc[k] += w[k]
        pick = max((k for k in acc if n[k] < cap[k]), key=lambda k: acc[k])
        acc[pick] -= 1.0
        n[pick] += 1
        copier = None
        if pick == "pool":
            cp_acc += N_POOLCP_ACT / float(N_POOL)
            if cp_acc >= 1.0 and cp_n < N_POOLCP_ACT:
                cp_acc -= 1.0
                cp_n += 1
                copier = "act"
            else:
                copier = "dve"
        plan.append((pick, copier))
    return plan


def build_nc():
    nc = bacc.Bacc()
    xT = nc.declare_dram_parameter("xT", [C + 1, T], BF16, isOutput=False)
    wq = nc.declare_dram_parameter("wq", [C, H * HS], BF16, isOutput=False)
    wk = nc.declare_dram_parameter("wk", [C, H * HS], BF16, isOutput=False)
    wv = nc.declare_dram_parameter("wv", [C + 1, H * 33], BF16, isOutput=False)
    wpa = nc.declare_dram_parameter("wpa", [97, C], BF16, isOutput=False)
    wpb = nc.declare_dram_parameter("wpb", [96, C], BF16, isOutput=False)
    idn = nc.declare_dram_parameter("idn", [P, P], F32, isOutput=False)
    out = nc.declare_dram_parameter("out", [T, C], F32, isOutput=True)

    plan = _exp_plan()

    with tile.TileContext(nc) as tc, ExitStack() as ctx:
        singles = ctx.enter_context(tc.tile_pool(name="singles", bufs=1))
        qk_pool = ctx.enter_context(tc.tile_pool(name="qk", bufs=1))
        vaug_pool = ctx.enter_context(tc.tile_pool(name="vaug", bufs=1))
        pt_pool = ctx.enter_context(tc.tile_pool(name="ptp", bufs=12))
        stg_pool = ctx.enter_context(tc.tile_pool(name="stg", bufs=4))
        on_pool = ctx.enter_context(tc.tile_pool(name="onp", bufs=4))
        ot_pool = ctx.enter_context(tc.tile_pool(name="otp", bufs=2))
        small = ctx.enter_context(tc.tile_pool(name="small", bufs=4))
        ysb_pool = ctx.enter_context(tc.tile_pool(name="ysb", bufs=3))

        # ---------------- load inputs ----------------
        xa = singles.tile([P, T], BF16)
        nc.sync.dma_start(xa, xT[0:P, :])
        xb = singles.tile([C + 1 - P, T], BF16)          # 65 rows (ones last)
        nc.sync.dma_start(xb, xT[P:C + 1, :])

        w_sb = {}
        for name, dram in (("q", wq), ("k", wk)):
            a = singles.tile([P, H * HS], BF16, name=f"w{name}a")
            nc.sync.dma_start(a, dram[0:P, :])
            b = singles.tile([C - P, H * HS], BF16, name=f"w{name}b")
            nc.sync.dma_start(b, dram[P:C, :])
            w_sb[name] = (a, b)
        wva = singles.tile([P, H * 33], BF16)
        nc.sync.dma_start(wva, wv[0:P, :])
        wvb = singles.tile([C + 1 - P, H * 33], BF16)
        nc.sync.dma_start(wvb, wv[P:C + 1, :])
        wpa_sb = singles.tile([97, C], BF16)
        nc.sync.dma_start(wpa_sb, wpa[:, :])
        wpb_sb = singles.tile([96, C], BF16)
        nc.sync.dma_start(wpb_sb, wpb[:, :])
        idn_sb = singles.tile([P, P], F32)
        nc.sync.dma_start(idn_sb, idn[:, :])

        # ---------------- phase 1: qT, kT, v_aug ----------------
        qT_a = qk_pool.tile([P, T], BF16)       # heads 0..3, d-major
        qT_b = qk_pool.tile([C - P, T], BF16)   # heads 4,5
        kT_a = qk_pool.tile([P, T], BF16)
        kT_b = qk_pool.tile([C - P, T], BF16)
        v_aug = []
        with tc.tile_pool(name="pqk", bufs=2, space="PSUM") as pqk, \
             tc.tile_pool(name="pv", bufs=2, space="PSUM") as pv:
            for proj, dst_a, dst_b in (("q", qT_a, qT_b), ("k", kT_a, kT_b)):
                wa, wb = w_sb[proj]
                for dlo, dsz, dst in ((0, P, dst_a), (P, C - P, dst_b)):
                    for t0 in range(0, T, TCH):
                        ps = pqk.tile([P, TCH], F32, name="psq", tag="psq")
                        nc.tensor.matmul(
                            ps[0:dsz, :], wa[:, dlo:dlo + dsz],
                            xa[:, t0:t0 + TCH], start=True, stop=False)
                        nc.tensor.matmul(
                            ps[0:dsz, :], wb[:, dlo:dlo + dsz],
                            xb[0:C - P, t0:t0 + TCH], start=False, stop=True)
                        nc.scalar.activation(
                            dst[0:dsz, t0:t0 + TCH], ps[0:dsz, :], Copy,
                            scale=1.0)
            for si in range(NS):
                s0 = si * P
                ps = pv.tile([P, H * 33], F32, name="psv", tag="psv")
                nc.tensor.matmul(ps, xa[:, s0:s0 + P], wva,
                                 start=True, stop=False)
                nc.tensor.matmul(ps, xb[:, s0:s0 + P], wvb,
                                 start=False, stop=True)
                va = vaug_pool.tile([P, H * 33], BF16,
                                    name=f"vaug{si}", tag=f"vaug{si}")
                nc.vector.tensor_copy(va, ps)
                v_aug.append(va)

        # ---------------- phase 2: attention ----------------
        def hsrc(h):
            if h < 4:
                return kT_a, qT_a, HS * h
            return kT_b, qT_b, HS * (h - 4)

        exp_i = [0]

        with (
            tc.tile_pool(name="pst", bufs=3, space="PSUM") as pst_pool,
            tc.tile_pool(name="pav", bufs=1, space="PSUM") as pav_pool,
        ):
            for tci, tc0 in enumerate(range(0, T, TCH)):
                # two av accumulators, each one PSUM bank: cols =
                # (tt%2)*198 + h*33 + [0..32]; col 32 of each head = rowsum
                av = [pav_pool.tile([P, 396], F32, name=f"av{b}", tag=f"av{b}")
                      for b in range(2)]
                pend = {}

                def issue_av(si):
                    for tt in range(4):
                        b, jj = tt // 2, tt % 2
                        for h in range(H):
                            p, half = h // 2, h % 2
                            ptp = pend[si][p]
                            nc.tensor.matmul(
                                av[b][:, jj * 198 + h * 33:
                                      jj * 198 + (h + 1) * 33],
                                ptp[:, half * TCH + tt * P:
                                    half * TCH + tt * P + P],
                                v_aug[si][:, h * 33:(h + 1) * 33],
                                # start=True marks the WHOLE psum bank
                                # pending-zero, so only the first chain into
                                # each bank may set it; the other chains'
                                # first writes then overwrite pending-zero
                                # bytes (= implicit zero init).
                                start=(si == 0 and jj == 0 and h == 0),
                                stop=(si == NS - 1),
                                skip_group_check=True)

                for si in range(NS):
                    s0 = si * P
                    if si >= 2:
                        issue_av(si - 2)
                    cur = []
                    for p in range(H // 2):
                        hA, hB = 2 * p, 2 * p + 1
                        stp = pst_pool.tile([P, 2 * TCH], F32,
                                            name="stp", tag="stp")
                        for half, h in ((0, hA), (1, hB)):
                            kT_t, qT_t, pb = hsrc(h)
                            nc.tensor.matmul(
                                stp[:, half * TCH:(half + 1) * TCH],
                                kT_t[pb:pb + HS, s0:s0 + P],
                                qT_t[pb:pb + HS, tc0:tc0 + TCH],
                                start=True, stop=True, tile_position=(pb, 0))
                        eng, copier = plan[exp_i[0]]
                        exp_i[0] += 1
                        ptp = pt_pool.tile([P, 2 * TCH], BF16,
                                           name="ptp", tag="ptp")
                        if eng == "act":
                            nc.scalar.activation(ptp, stp, Exp, scale=SCALE)
                        elif eng == "dve":
                            nc.vector.tensor_scalar(
                                ptp.bitcast(I16), stp, SCH_A * SCALE, SCH_B,
                                op0=MUL, op1=ADD)
                        else:
                            stg = stg_pool.tile([P, 2 * TCH], F16,
                                                name="stg", tag="stg")
                            if copier == "act":
                                nc.scalar.activation(stg, stp, Copy,
                                                     scale=SCALE)
                            else:
                                nc.vector.tensor_scalar_mul(stg, stp, SCALE)
                            nc.gpsimd.tensor_scalar(
                                ptp.bitcast(I16), stg, SCH_A, SCH_B,
                                op0=MUL, op1=ADD)
                        cur.append(ptp)
                    pend[si] = cur
                issue_av(NS - 2)
                issue_av(NS - 1)

                # ---- tail: normalize all 4 t-tiles first (frees av banks)
                rrecs = []
                ons = []
                for tt in range(4):
                    b, off = tt // 2, (tt % 2) * 198
                    avr = av[b][:, off:off + 198].rearrange(
                        "p (h e) -> p h e", h=H)
                    rrec = small.tile([P, H], F32, name="rrec", tag="rrec")
                    nc.vector.reciprocal(rrec[:, :, None], avr[:, :, 32:33])
                    on = on_pool.tile([P, 193], F32, name="on", tag="on")
                    for g in range(2):
                        og = 97 * g
                        nc.vector.tensor_tensor(
                            on[:, og:og + 96].rearrange(
                                "p (h e) -> p h e", h=3),
                            avr[:, 3 * g:3 * g + 3, 0:32],
                            rrec[:, 3 * g:3 * g + 3, None].to_broadcast(
                                (P, 3, 32)),
                            op=MUL)
                    nc.gpsimd.memset(on[:, 96:97], 1.0)
                    rrecs.append(rrec)
                    ons.append(on)

                # ---- transpose + project, reusing the freed av banks
                # av[0] holds three 128-col transpose slots (rotating),
                # av[1] holds two 192-col projection-psum slots.
                for tt in range(4):
                    on = ons[tt]
                    ca = 128 * ((2 * tt) % 3)
                    cb = 128 * ((2 * tt + 1) % 3)
                    ga = av[0][:, ca:ca + 128]
                    gb = av[0][:, cb:cb + 128]
                    nc.tensor.transpose(ga[0:97, :], on[:, 0:97], idn_sb)
                    ota = ot_pool.tile([97, P], BF16, name="ota", tag="ota")
                    nc.scalar.activation(ota, ga[0:97, :], Copy, scale=1.0)
                    nc.tensor.transpose(gb[0:96, :], on[:, 97:193], idn_sb)
                    otb = ot_pool.tile([96, P], BF16, name="otb", tag="otb")
                    nc.vector.tensor_copy(otb, gb[0:96, :])
                    py = av[1][:, (tt % 2) * 192:(tt % 2) * 192 + 192]
                    nc.tensor.matmul(py, ota, wpa_sb, start=True, stop=False,
                                     skip_group_check=True)
                    nc.tensor.matmul(py, otb, wpb_sb, start=False, stop=True,
                                     skip_group_check=True)
                    ysb = ysb_pool.tile([P, C], F32, name="ysbt", tag="ysbt")
                    nc.vector.tensor_copy(ysb, py)
                    nc.sync.dma_start(out[tc0 + tt * P:tc0 + (tt + 1) * P, :],
                                      ysb)

    nc.compile()
    return nc


def _get_nc():
    if "nc" not in _CACHE:
        _CACHE["nc"] = build_nc()
    return _CACHE["nc"]


def make_in_maps(x, Wq, Wk, Wv, Wproj, bproj):
    bf = ml_dtypes.bfloat16
    x = np.asarray(x, np.float32)
    pack = lambda w: np.ascontiguousarray(
        np.transpose(np.asarray(w, np.float32), (1, 0, 2)).reshape(C, H * HS)
    ).astype(bf)
    wq, wk = pack(Wq), pack(Wk)

    wv_aug = np.zeros((C + 1, H * 33), np.float32)
    Wv = np.asarray(Wv, np.float32)
    for h in range(H):
        wv_aug[0:C, h * 33:h * 33 + 32] = Wv[h]
        wv_aug[C, h * 33 + 32] = 1.0
    wv_aug = wv_aug.astype(bf)

    Wp = np.asarray(Wproj, np.float32)          # [H*HS, C]
    wpa = np.zeros((97, C), np.float32)
    wpa[0:96] = Wp[0:96]
    wpa[96] = np.asarray(bproj, np.float32)
    wpb = Wp[96:192]
    wpa = wpa.astype(bf)
    wpb = np.ascontiguousarray(wpb).astype(bf)

    idn = np.eye(P, dtype=np.float32)

    maps = []
    for i in range(B):
        xp = np.ones((C + 1, T), np.float32)
        xp[0:C] = x[i].T
        maps.append({"xT": xp.astype(bf), "wq": wq, "wk": wk,
                     "wv": wv_aug, "wpa": wpa, "wpb": wpb, "idn": idn})
    return maps


def run(inputs, trace=False, **kw):
    nc = _get_nc()
    in_maps = make_in_maps(**inputs)
    res = run_bass_kernel_spmd(nc, in_maps, core_ids=list(range(B)),
                               trace=trace, **kw)
    y = np.stack([np.asarray(res.results[i]["out"], np.float32)
                  for i in range(B)], axis=0)
    return y, res


def kernel(**inputs):
    y, _ = run(inputs, trace=False)
    return y
